# revision 40
# baseline (speedup 1.0000x reference)
"""Additive (Bahdanau) attention on 8 TRN2 NeuronCores.

Problem: B=8, LQ=256, LK=1024, DQ=DK=DV=512, H=128.
  q = Q @ W_q; k = K @ W_k
  scores[b,q,k] = sum_h w_v[h] * tanh(qf[b,q,h] + kf[b,k,h])
  out = softmax_k(mask(scores)) @ V

Sharding: data-parallel over QUERIES — core c computes query rows
[32c, 32c+32) of every batch, so every core's workload is identically
sum_b 32 * valid_len_b regardless of the length distribution, with no
cross-core communication (softmax rows live entirely on one core).
The graph is compiled per call from the actual valid_lengths: the
tanh/score/softmax path runs at the EXACT valid length (no masked key
column is ever computed, which also removes the mask entirely), while
the K-transpose / V / attn@V paths use 128-granular extents.

Per-core pipeline (h=H=128 on SBUF partitions):
  - Per batch: K rows are PE-transposed and projected; kfT_b is
    DVE-copied to SBUF so PSUM stays free and later batches' K paths
    overlap the tanh stream.
  - Per query: one ScalarE instruction T = tanh(kfT + qf_col) with the
    activation unit's free per-partition bias add (FD = valid_len), and
    ceil(len/512) m=1 TensorE matmuls with stationary w_v reduce over
    h. Matmul PSUM rows must start at 32-aligned partitions, so 4
    queries per PSUM generation land at partitions {0,32,64,96}
    (concurrent PE column groups), are DVE-copied to SBUF, and gathered
    into compact partitions by a strided SBUF->SBUF DMA. A computed
    fraction of queries evaluates tanh on the otherwise-idle VectorE
    instead (clamped degree-9 odd polynomial) to balance engine load.
  - Softmax: exp on ScalarE emits row sums via accum_out;
    normalization is deferred to one per-partition multiply on the
    output. The epilogue of batch b is issued after batch b+1's score
    loop so ScalarE's FIFO never stalls on the gather DMAs.
  - attn @ V over valid key chunks (ragged last chunk); DMA out.
  - Queues: startup-critical loads on the SP HWDGE queue, bulk/
    non-critical DMAs on the gpsimd SWDGE queue.

ScalarE is the roofline: ~sum_b 32*(222+len_b) cycles @ 1.2 GHz minus
the VectorE offload; cost-model exec time 171.1 us for the reference
lengths (vs ~300 us for the naive batch-parallel full-length version).
"""

import sys

if "/opt/trn_rl_repo" not in sys.path:
    sys.path.insert(0, "/opt/trn_rl_repo")

import numpy as np
import ml_dtypes

import concourse.mybir as mybir
from concourse import tile, bacc
from concourse.bass_utils import run_bass_kernel_spmd
B, LQ, LK, DQ, DK, DV, H = 8, 256, 1024, 512, 512, 512, 128
N_CORES = 8
QPC = LQ // N_CORES  # 32 query rows per core per batch
MASK_NEG = -50.0  # scores bounded by sum|w_v| ~ 9, so -50 is "minus inf"

_BF16 = mybir.dt.bfloat16
_F32 = mybir.dt.float32

# Degree-9 odd minimax-ish polynomial for tanh on [-3.6, 3.6] (max err
# ~0.012; inputs are clamped to that range first, clamp error <= 1.5e-3).
# Used only for the small fraction of queries whose tanh is offloaded
# from ScalarE to the otherwise-idle VectorE.
_TANH_CLAMP = 3.6
_TANH_C = (0.95397023, -0.21573944, 0.032842446, -0.0024608947, 6.959084e-05)

_cached = {}


def _plan_offload(extents):
    """Pick which (batch, query) pairs compute tanh on VectorE so ACT and
    DVE busy-times balance (with margin). Returns set of (b, i)."""
    act = {b: (222 + e) / 1.2 for b, e in enumerate(extents)}  # ns per query
    dve = {b: (4.0 * e + 11 * 58) / 0.96 for b, e in enumerate(extents)}
    act_total = sum(32 * act[b] for b in range(len(extents))) + 8000.0  # + exps
    # baseline DVE busy: gather copies + kf/qT copies + mask adds (approx)
    dve_total = sum(8 * (120 + e) / 0.96 for e in extents) + 35000.0
    off = set()
    margin = 9500.0
    i_per_b = {b: 0 for b in range(len(extents))}
    while True:
        cand = min(range(len(extents)), key=lambda b: (i_per_b[b], -extents[b]))
        b = cand
        if i_per_b[b] >= 16:
            break
        new_act = act_total - act[b]
        new_dve = dve_total + dve[b]
        if new_dve + margin >= new_act:
            break
        perm = (0, 8, 16, 24, 4, 12, 20, 28, 2, 10, 18, 26, 6, 14, 22, 30)
        off.add((b, perm[i_per_b[b]]))  # spread offloads across generations
        i_per_b[b] += 1
        act_total, dve_total = new_act, new_dve
    return off


def _build(lens):
    """Build the SPMD graph for one core given exact per-batch valid
    lengths (each in [1, 1024]). The K-transpose / V / attn@V paths use
    128-granular extents; the tanh/score/softmax path uses the exact
    lengths (no masked column is ever computed, so no mask is needed)."""
    nc = bacc.Bacc("TRN2", target_bir_lowering=False, debug=False)

    lens = [int(l) for l in lens]
    extents = [max(128, ((l + 127) // 128) * 128) for l in lens]
    total_k = int(sum(extents))
    Qp = nc.declare_dram_parameter("Q", [B * QPC, DQ], _BF16, isOutput=False)
    Kp = nc.declare_dram_parameter("K", [total_k, DK], _BF16, isOutput=False)
    Vp = nc.declare_dram_parameter("V", [total_k, DV], _BF16, isOutput=False)
    Wqp = nc.declare_dram_parameter("Wq", [DQ, H], _BF16, isOutput=False)
    Wkp = nc.declare_dram_parameter("Wk", [DK, H], _BF16, isOutput=False)
    wvp = nc.declare_dram_parameter("wv", [H, 1], _BF16, isOutput=False)
    idp = nc.declare_dram_parameter("ident", [128, 128], _BF16, isOutput=False)
    outp = nc.declare_dram_parameter("out", [B, QPC, DV], _F32, isOutput=True)

    NDQ = DQ // 128  # 4 contraction chunks for the projections
    offs = np.concatenate([[0], np.cumsum(extents)]).astype(int)
    offload = _plan_offload(lens)

    with tile.TileContext(nc) as tc:
        with (
            tc.tile_pool(name="const", bufs=1) as const,
            tc.tile_pool(name="nat", bufs=4) as nat,
            tc.tile_pool(name="kv", bufs=3) as kv,
            tc.tile_pool(name="tpool", bufs=14) as tpool,
            tc.tile_pool(name="tpoly", bufs=3) as tpoly,
            tc.tile_pool(name="spool", bufs=6) as spool,
            tc.tile_pool(name="softm", bufs=2) as softm,
            tc.tile_pool(name="ps_gen", bufs=2, space="PSUM") as ps_gen,
            tc.tile_pool(name="ps_kp", bufs=2, space="PSUM") as ps_kp,
            tc.tile_pool(name="ps_tail", bufs=2, space="PSUM") as ps_tail,
        ):
            # ---- constants / weights -------------------------------------
            wq_sb = const.tile([128, NDQ, H], _BF16)
            nc.gpsimd.dma_start(out=wq_sb, in_=Wqp[:, :].rearrange("(c p) h -> p c h", p=128))
            wk_sb = const.tile([128, NDQ, H], _BF16)
            nc.gpsimd.dma_start(out=wk_sb, in_=Wkp[:, :].rearrange("(c p) h -> p c h", p=128))
            wv_sb = const.tile([H, 1], _BF16)
            nc.gpsimd.dma_start(out=wv_sb, in_=wvp[:, :])
            ident = const.tile([128, 128], _BF16)
            nc.sync.dma_start(out=ident, in_=idp[:, :])

            # ---- qfT (h, B*QPC) for this core's queries ------------------
            qT_sb = const.tile([128, NDQ, B * QPC], _BF16)
            for qt in range(B * QPC // 128):
                qn = nat.tile([128, DQ], _BF16, tag="nat")
                nc.sync.dma_start(out=qn, in_=Qp[qt * 128 : (qt + 1) * 128, :])
                pst = ps_kp.tile([128, 512], _BF16, tag="kp")
                for dc in range(NDQ):
                    nc.tensor.transpose(pst[:, dc * 128 : (dc + 1) * 128], qn[:, dc * 128 : (dc + 1) * 128], ident)
                nc.vector.tensor_copy(
                    qT_sb[:, :, qt * 128 : (qt + 1) * 128],
                    pst.rearrange("p (c x) -> p c x", c=NDQ),
                )
            qf_ps = ps_tail.tile([128, B * QPC], _F32, tag="tail")
            for dc in range(NDQ):
                nc.tensor.matmul(
                    out=qf_ps,
                    lhsT=wq_sb[:, dc, :],
                    rhs=qT_sb[:, dc, :],
                    start=(dc == 0),
                    stop=(dc == NDQ - 1),
                )
            qfT_sb = const.tile([128, B * QPC], _F32)
            nc.vector.tensor_copy(qfT_sb, qf_ps)

            # ---- helpers --------------------------------------------------
            def k_path(b):
                """K transpose + projection; kfT_b lands in SBUF (f32)."""
                ext = int(extents[b])
                nkc = ext // 128
                o0 = int(offs[b])
                kT_b = kv.tile([128, NDQ, ext], _BF16, tag="kT")
                for kc in range(nkc):
                    kn = nat.tile([128, DK], _BF16, tag="nat")
                    nc.sync.dma_start(out=kn, in_=Kp[o0 + kc * 128 : o0 + (kc + 1) * 128, :])
                    pst = ps_kp.tile([128, 512], _BF16, tag="kp")
                    for dc in range(NDQ):
                        nc.tensor.transpose(pst[:, dc * 128 : (dc + 1) * 128], kn[:, dc * 128 : (dc + 1) * 128], ident)
                    nc.vector.tensor_copy(
                        kT_b[:, :, kc * 128 : (kc + 1) * 128],
                        pst.rearrange("p (c x) -> p c x", c=NDQ),
                    )
                kf_sb = kv.tile([128, ext], _F32, tag="kf")
                for c0 in range(0, ext, 512):
                    cn = min(512, ext - c0)
                    kf_ps = ps_kp.tile([128, 512], _F32, tag="kp")
                    for dc in range(NDQ):
                        nc.tensor.matmul(
                            out=kf_ps[:, 0:cn],
                            lhsT=wk_sb[:, dc, :],
                            rhs=kT_b[:, dc, c0 : c0 + cn],
                            start=(dc == 0),
                            stop=(dc == NDQ - 1),
                        )
                    nc.vector.tensor_copy(kf_sb[:, c0 : c0 + cn], kf_ps[:, 0:cn])
                v_b = kv.tile([128, nkc, DV], _BF16, tag="v")
                nc.gpsimd.dma_start(
                    out=v_b, in_=Vp[o0 : o0 + ext, :].rearrange("(c p) d -> p c d", p=128)
                )
                return kf_sb, v_b

            def scores(b, kf_sb, last=False):
                """tanh + m=1 reduce matmuls + gather for batch b (exact
                valid length: no masked column is ever computed)."""
                ln = int(lens[b])
                nchunks = [(c0, min(512, ln - c0)) for c0 in range(0, ln, 512)]
                s_b = spool.tile([QPC, ln], _F32, tag="s")
                for g in range(QPC // 4):
                    sg = ps_gen.tile([128, ln], _F32, tag="gen")
                    for j in range(4):
                        q = b * QPC + g * 4 + j
                        p = 32 * j
                        t_q = tpool.tile([128, ln], _BF16, tag="t")
                        if (b, g * 4 + j) in offload:
                            # polynomial tanh on VectorE (frees ScalarE time)
                            AL = mybir.AluOpType
                            c0_, c1_, c2_, c3_, c4_ = _TANH_C
                            tx = tpoly.tile([128, ln], _BF16, tag="tx")
                            nc.vector.tensor_scalar(
                                out=tx, in0=kf_sb[:, 0:ln], scalar1=qfT_sb[:, q : q + 1],
                                scalar2=_TANH_CLAMP, op0=AL.add, op1=AL.min,
                            )
                            nc.vector.tensor_scalar(
                                out=tx, in0=tx, scalar1=-_TANH_CLAMP, scalar2=None,
                                op0=AL.max,
                            )
                            tu = tpoly.tile([128, ln], _BF16, tag="tu")
                            nc.vector.tensor_tensor(out=tu, in0=tx, in1=tx, op=AL.mult)
                            tw = tpoly.tile([128, ln], _BF16, tag="tw")
                            nc.vector.tensor_scalar(
                                out=tw, in0=tu, scalar1=c4_, scalar2=None, op0=AL.mult
                            )
                            for cc in (c3_, c2_, c1_):
                                nc.vector.tensor_scalar(
                                    out=tw, in0=tw, scalar1=cc, scalar2=None, op0=AL.add
                                )
                                nc.vector.tensor_tensor(out=tw, in0=tw, in1=tu, op=AL.mult)
                            nc.vector.tensor_scalar(
                                out=tw, in0=tw, scalar1=c0_, scalar2=None, op0=AL.add
                            )
                            nc.vector.tensor_tensor(out=t_q, in0=tw, in1=tx, op=AL.mult)
                        else:
                            nc.scalar.activation(
                                out=t_q,
                                in_=kf_sb[:, 0:ln],
                                func=mybir.ActivationFunctionType.Tanh,
                                bias=qfT_sb[:, q : q + 1],
                                scale=1.0,
                            )
                        for c0, cn in nchunks:
                            nc.tensor.matmul(
                                out=sg[p : p + 1, c0 : c0 + cn],
                                lhsT=wv_sb,
                                rhs=t_q[:, c0 : c0 + cn],
                                start=True,
                                stop=True,
                                skip_group_check=True,
                                tile_position=(0, p),
                            )
                    stg = spool.tile([128, ln], _F32, tag="stg")
                    nc.vector.tensor_copy(stg, sg)
                    eng = nc.sync if last else nc.gpsimd
                    eng.dma_start(out=s_b[g * 4 : g * 4 + 4, :], in_=stg[0:128:32, :])
                return s_b

            def epilogue(b, s_b, v_b):
                """softmax + attn @ V + output DMA for batch b. Keys beyond
                the valid length were never computed, so no mask is needed
                (exactly matching the reference's masked softmax)."""
                ln = int(lens[b])
                nkc = (ln + 127) // 128
                e_b = softm.tile([QPC, ln], _BF16, tag="e")
                rsum = softm.tile([QPC, 1], _F32, tag="rs")
                nc.scalar.activation(
                    out=e_b, in_=s_b, func=mybir.ActivationFunctionType.Exp, accum_out=rsum
                )
                rinv = softm.tile([QPC, 1], _F32, tag="ri")
                nc.vector.reciprocal(rinv, rsum)

                eT_b = softm.tile([128, nkc * QPC], _BF16, tag="et")
                for g4 in range(0, nkc, 4):
                    gn = min(4, nkc - g4)
                    pst = ps_tail.tile([128, 4 * QPC], _BF16, tag="tail")
                    for j in range(gn):
                        kc = g4 + j
                        r = min(128, ln - kc * 128)
                        nc.tensor.transpose(
                            pst[0:r, j * QPC : (j + 1) * QPC],
                            e_b[:, kc * 128 : kc * 128 + r],
                            ident[0:QPC, 0:QPC],
                        )
                    nc.vector.tensor_copy(
                        eT_b[:, g4 * QPC : (g4 + gn) * QPC], pst[:, 0 : gn * QPC]
                    )

                o_ps = ps_tail.tile([QPC, DV], _F32, tag="tail")
                for kc in range(nkc):
                    r = min(128, ln - kc * 128)
                    nc.tensor.matmul(
                        out=o_ps,
                        lhsT=eT_b[0:r, kc * QPC : (kc + 1) * QPC],
                        rhs=v_b[0:r, kc, :],
                        start=(kc == 0),
                        stop=(kc == nkc - 1),
                    )
                osb = softm.tile([QPC, DV], _F32, tag="o")
                nc.vector.tensor_scalar_mul(osb, o_ps, rinv[:, 0:1])
                nc.sync.dma_start(out=outp[b, :, :], in_=osb)

            # ---- software-pipelined batch loop ---------------------------
            # epilogue(b) is issued after scores(b+1) so ScalarE's exp never
            # blocks the next batch's tanh stream waiting on gather DMAs.
            # Batch order: 2nd-smallest first (short pipeline fill), smallest
            # last (short drain: its gather chain and epilogue set the tail).
            asc = sorted(range(B), key=lambda b: (int(lens[b]), b))
            batch_order = [asc[1]] + asc[2:][::-1] + [asc[0]]
            pending = None
            for bi, b in enumerate(batch_order):
                kf_sb, v_b = k_path(b)
                s_b = scores(b, kf_sb, last=(bi == B - 1))
                if pending is not None:
                    epilogue(*pending)
                pending = (b, s_b, v_b)
            epilogue(*pending)

    nc.finalize()
    return nc


def _get_nc(extents):
    key = tuple(int(e) for e in extents)
    if key not in _cached:
        _cached[key] = _build(key)
    return _cached[key]


def kernel(Q, K, V, valid_lengths, W_q, W_k, w_v, _want_trace=False):
    Q = np.asarray(Q, dtype=np.float32)
    K = np.asarray(K, dtype=np.float32)
    V = np.asarray(V, dtype=np.float32)
    vl = np.asarray(valid_lengths).astype(np.int64).reshape(B)
    W_q = np.asarray(W_q, dtype=np.float32)
    W_k = np.asarray(W_k, dtype=np.float32)
    w_v = np.asarray(w_v, dtype=np.float32)

    lens = np.clip(vl, 1, LK)
    extents = np.clip(np.ceil(lens / 128.0).astype(int) * 128, 128, LK)
    nc = _get_nc(lens)

    bf = ml_dtypes.bfloat16
    Kc = np.concatenate([K[b, : extents[b], :] for b in range(B)], axis=0).astype(bf)
    Vc = np.concatenate([V[b, : extents[b], :] for b in range(B)], axis=0).astype(bf)
    Wqb = W_q.astype(bf)
    Wkb = W_k.astype(bf)
    wvb = w_v.reshape(H, 1).astype(bf)
    Qb = Q.astype(bf)

    in_maps = []
    for c in range(N_CORES):
        Qcore = np.concatenate(
            [Qb[b, c * QPC : (c + 1) * QPC, :] for b in range(B)], axis=0
        )
        in_maps.append(
            {
                "Q": Qcore,
                "K": Kc,
                "V": Vc,
                "Wq": Wqb,
                "Wk": Wkb,
                "wv": wvb,
                "ident": np.eye(128, dtype=bf),
            }
        )

    kwargs = {"trace": True} if _want_trace else {}
    res = run_bass_kernel_spmd(nc, in_maps, core_ids=list(range(N_CORES)), **kwargs)
    out = np.empty((B, LQ, DV), dtype=np.float32)
    for c in range(N_CORES):
        oc = res.results[c]["out"]  # (B, QPC, DV)
        for b in range(B):
            out[b, c * QPC : (c + 1) * QPC, :] = oc[b]
    if _want_trace:
        _cached["last_result"] = res
    return out


# revision 43
# speedup vs baseline: 1.0215x; 1.0215x over previous
"""Additive (Bahdanau) attention on 8 TRN2 NeuronCores.

Problem: B=8, LQ=256, LK=1024, DQ=DK=DV=512, H=128.
  q = Q @ W_q; k = K @ W_k
  scores[b,q,k] = sum_h w_v[h] * tanh(qf[b,q,h] + kf[b,k,h])
  out = softmax_k(mask(scores)) @ V

Sharding: data-parallel over QUERIES — core c computes query rows
[32c, 32c+32) of every batch, so every core's workload is identically
sum_b 32 * valid_len_b regardless of the length distribution, with no
cross-core communication (softmax rows live entirely on one core).
The graph is compiled per call from the actual valid_lengths: the
tanh/score/softmax path runs at the EXACT valid length (no masked key
column is ever computed, which also removes the mask entirely), while
the K-transpose / V / attn@V paths use 128-granular extents.

Per-core pipeline (h=H=128 on SBUF partitions):
  - Per batch: K rows are PE-transposed and projected; kfT_b is
    DVE-copied to SBUF so PSUM stays free and later batches' K paths
    overlap the tanh stream.
  - Per query: one ScalarE instruction T = tanh(kfT + qf_col) with the
    activation unit's free per-partition bias add (FD = valid_len), and
    ceil(len/512) m=1 TensorE matmuls with stationary w_v reduce over
    h. Matmul PSUM rows must start at 32-aligned partitions, so 4
    queries per PSUM generation land at partitions {0,32,64,96}
    (concurrent PE column groups), are DVE-copied to SBUF, and gathered
    into compact partitions by a strided SBUF->SBUF DMA. A computed
    fraction of queries evaluates tanh on the otherwise-idle VectorE
    instead (clamped degree-9 odd polynomial) to balance engine load.
  - Softmax: exp on ScalarE emits row sums via accum_out;
    normalization is deferred to one per-partition multiply on the
    output. The epilogue of batch b is issued after batch b+1's score
    loop so ScalarE's FIFO never stalls on the gather DMAs.
  - attn @ V over valid key chunks (ragged last chunk); DMA out.
  - Queues: startup-critical loads on the SP HWDGE queue, bulk/
    non-critical DMAs on the gpsimd SWDGE queue.

ScalarE is the roofline: ~sum_b 32*(222+len_b) cycles @ 1.2 GHz minus
the VectorE offload; cost-model exec time 171.1 us for the reference
lengths (vs ~300 us for the naive batch-parallel full-length version).
"""

import sys

if "/opt/trn_rl_repo" not in sys.path:
    sys.path.insert(0, "/opt/trn_rl_repo")

import numpy as np
import ml_dtypes

import concourse.mybir as mybir
from concourse import tile, bacc
from concourse.bass_utils import run_bass_kernel_spmd
B, LQ, LK, DQ, DK, DV, H = 8, 256, 1024, 512, 512, 512, 128
N_CORES = 8
QPC = LQ // N_CORES  # 32 query rows per core per batch
MASK_NEG = -50.0  # scores bounded by sum|w_v| ~ 9, so -50 is "minus inf"

_BF16 = mybir.dt.bfloat16
_F32 = mybir.dt.float32

# Degree-9 odd minimax-ish polynomial for tanh on [-3.6, 3.6] (max err
# ~0.012; inputs are clamped to that range first, clamp error <= 1.5e-3).
# Used only for the small fraction of queries whose tanh is offloaded
# from ScalarE to the otherwise-idle VectorE.
_TANH_CLAMP = 3.6
_TANH_C = (0.95397023, -0.21573944, 0.032842446, -0.0024608947, 6.959084e-05)

_cached = {}


def _plan_offload(extents):
    """Pick which (batch, query) pairs compute tanh on VectorE so ACT and
    DVE busy-times balance (with margin). Returns set of (b, i)."""
    act = {b: (222 + e) / 1.2 for b, e in enumerate(extents)}  # ns per query
    dve = {b: (4.0 * e + 11 * 58) / 0.96 for b, e in enumerate(extents)}
    act_total = sum(32 * act[b] for b in range(len(extents))) + 8000.0  # + exps
    # baseline DVE busy: gather copies + kf/qT copies + mask adds (approx)
    dve_total = sum(8 * (120 + e) / 0.96 for e in extents) + 35000.0
    off = set()
    margin = 9500.0
    i_per_b = {b: 0 for b in range(len(extents))}
    while True:
        cand = min(range(len(extents)), key=lambda b: (i_per_b[b], -extents[b]))
        b = cand
        if i_per_b[b] >= 16:
            break
        new_act = act_total - act[b]
        new_dve = dve_total + dve[b]
        if new_dve + margin >= new_act:
            break
        perm = (0, 8, 16, 24, 4, 12, 20, 28, 2, 10, 18, 26, 6, 14, 22, 30)
        off.add((b, perm[i_per_b[b]]))  # spread offloads across generations
        i_per_b[b] += 1
        act_total, dve_total = new_act, new_dve
    return off


def _build(lens):
    """Build the SPMD graph for one core given exact per-batch valid
    lengths (each in [1, 1024]). The K-transpose / V / attn@V paths use
    128-granular extents; the tanh/score/softmax path uses the exact
    lengths (no masked column is ever computed, so no mask is needed)."""
    nc = bacc.Bacc("TRN2", target_bir_lowering=False, debug=False)

    lens = [int(l) for l in lens]
    extents = [max(128, ((l + 127) // 128) * 128) for l in lens]
    total_k = int(sum(extents))
    Qp = nc.declare_dram_parameter("Q", [B * QPC, DQ], _BF16, isOutput=False)
    Kp = nc.declare_dram_parameter("K", [total_k, DK], _BF16, isOutput=False)
    Vp = nc.declare_dram_parameter("V", [total_k, DV], _BF16, isOutput=False)
    Wqp = nc.declare_dram_parameter("Wq", [DQ, H], _BF16, isOutput=False)
    Wkp = nc.declare_dram_parameter("Wk", [DK, H], _BF16, isOutput=False)
    wvp = nc.declare_dram_parameter("wv", [H, 1], _BF16, isOutput=False)
    idp = nc.declare_dram_parameter("ident", [128, 128], _BF16, isOutput=False)
    outp = nc.declare_dram_parameter("out", [B, QPC, DV], _F32, isOutput=True)

    NDQ = DQ // 128  # 4 contraction chunks for the projections
    offs = np.concatenate([[0], np.cumsum(extents)]).astype(int)
    offload = _plan_offload(lens)

    with tile.TileContext(nc) as tc:
        with (
            tc.tile_pool(name="const", bufs=1) as const,
            tc.tile_pool(name="nat", bufs=4) as nat,
            tc.tile_pool(name="kv", bufs=3) as kv,
            tc.tile_pool(name="tpool", bufs=14) as tpool,
            tc.tile_pool(name="tpoly", bufs=3) as tpoly,
            tc.tile_pool(name="spool", bufs=6) as spool,
            tc.tile_pool(name="softm", bufs=2) as softm,
            tc.tile_pool(name="ps_gen", bufs=2, space="PSUM") as ps_gen,
            tc.tile_pool(name="ps_kp", bufs=2, space="PSUM") as ps_kp,
            tc.tile_pool(name="ps_tail", bufs=2, space="PSUM") as ps_tail,
        ):
            # ---- constants / weights -------------------------------------
            wq_sb = const.tile([128, NDQ, H], _BF16)
            nc.gpsimd.dma_start(out=wq_sb, in_=Wqp[:, :].rearrange("(c p) h -> p c h", p=128))
            wk_sb = const.tile([128, NDQ, H], _BF16)
            nc.gpsimd.dma_start(out=wk_sb, in_=Wkp[:, :].rearrange("(c p) h -> p c h", p=128))
            wv_sb = const.tile([H, 1], _BF16)
            nc.gpsimd.dma_start(out=wv_sb, in_=wvp[:, :])
            ident = const.tile([128, 128], _BF16)
            nc.sync.dma_start(out=ident, in_=idp[:, :])

            # ---- qfT (h, B*QPC) for this core's queries ------------------
            qT_sb = const.tile([128, NDQ, B * QPC], _BF16)
            for qt in range(B * QPC // 128):
                qn = nat.tile([128, DQ], _BF16, tag="nat")
                nc.sync.dma_start(out=qn, in_=Qp[qt * 128 : (qt + 1) * 128, :])
                pst = ps_kp.tile([128, 512], _BF16, tag="kp")
                for dc in range(NDQ):
                    nc.tensor.transpose(pst[:, dc * 128 : (dc + 1) * 128], qn[:, dc * 128 : (dc + 1) * 128], ident)
                nc.vector.tensor_copy(
                    qT_sb[:, :, qt * 128 : (qt + 1) * 128],
                    pst.rearrange("p (c x) -> p c x", c=NDQ),
                )
            qf_ps = ps_tail.tile([128, B * QPC], _F32, tag="tail")
            for dc in range(NDQ):
                nc.tensor.matmul(
                    out=qf_ps,
                    lhsT=wq_sb[:, dc, :],
                    rhs=qT_sb[:, dc, :],
                    start=(dc == 0),
                    stop=(dc == NDQ - 1),
                )
            qfT_sb = const.tile([128, B * QPC], _F32)
            nc.vector.tensor_copy(qfT_sb, qf_ps)

            # ---- helpers --------------------------------------------------
            def k_path(b):
                """K transpose + projection; kfT_b lands in SBUF (f32)."""
                ext = int(extents[b])
                nkc = ext // 128
                o0 = int(offs[b])
                kT_b = kv.tile([128, NDQ, ext], _BF16, tag="kT")
                for kc in range(nkc):
                    kn = nat.tile([128, DK], _BF16, tag="nat")
                    nc.sync.dma_start(out=kn, in_=Kp[o0 + kc * 128 : o0 + (kc + 1) * 128, :])
                    pst = ps_kp.tile([128, 512], _BF16, tag="kp")
                    for dc in range(NDQ):
                        nc.tensor.transpose(pst[:, dc * 128 : (dc + 1) * 128], kn[:, dc * 128 : (dc + 1) * 128], ident)
                    nc.vector.tensor_copy(
                        kT_b[:, :, kc * 128 : (kc + 1) * 128],
                        pst.rearrange("p (c x) -> p c x", c=NDQ),
                    )
                kf_sb = kv.tile([128, ext], _F32, tag="kf")
                for c0 in range(0, ext, 512):
                    cn = min(512, ext - c0)
                    kf_ps = ps_kp.tile([128, 512], _F32, tag="kp")
                    for dc in range(NDQ):
                        nc.tensor.matmul(
                            out=kf_ps[:, 0:cn],
                            lhsT=wk_sb[:, dc, :],
                            rhs=kT_b[:, dc, c0 : c0 + cn],
                            start=(dc == 0),
                            stop=(dc == NDQ - 1),
                        )
                    nc.vector.tensor_copy(kf_sb[:, c0 : c0 + cn], kf_ps[:, 0:cn])
                v_b = kv.tile([128, nkc, DV], _BF16, tag="v")
                nc.gpsimd.dma_start(
                    out=v_b, in_=Vp[o0 : o0 + ext, :].rearrange("(c p) d -> p c d", p=128)
                )
                return kf_sb, v_b

            def scores(b, kf_sb, last=False):
                """tanh + m=1 reduce matmuls + gather for batch b (exact
                valid length: no masked column is ever computed)."""
                ln = int(lens[b])
                nchunks = [(c0, min(512, ln - c0)) for c0 in range(0, ln, 512)]
                s_b = spool.tile([QPC, ln], _F32, tag="s")
                for g in range(QPC // 4):
                    sg = ps_gen.tile([128, ln], _F32, tag="gen")
                    for j in range(4):
                        q = b * QPC + g * 4 + j
                        p = 32 * j
                        t_q = tpool.tile([128, ln], _BF16, tag="t")
                        if (b, g * 4 + j) in offload:
                            # polynomial tanh on VectorE (frees ScalarE time)
                            AL = mybir.AluOpType
                            c0_, c1_, c2_, c3_, c4_ = _TANH_C
                            tx = tpoly.tile([128, ln], _BF16, tag="tx")
                            nc.vector.tensor_scalar(
                                out=tx, in0=kf_sb[:, 0:ln], scalar1=qfT_sb[:, q : q + 1],
                                scalar2=_TANH_CLAMP, op0=AL.add, op1=AL.min,
                            )
                            nc.vector.tensor_scalar(
                                out=tx, in0=tx, scalar1=-_TANH_CLAMP, scalar2=None,
                                op0=AL.max,
                            )
                            tu = tpoly.tile([128, ln], _BF16, tag="tu")
                            nc.vector.tensor_tensor(out=tu, in0=tx, in1=tx, op=AL.mult)
                            tw = tpoly.tile([128, ln], _BF16, tag="tw")
                            nc.vector.tensor_scalar(
                                out=tw, in0=tu, scalar1=c4_, scalar2=None, op0=AL.mult
                            )
                            for cc in (c3_, c2_, c1_):
                                nc.vector.tensor_scalar(
                                    out=tw, in0=tw, scalar1=cc, scalar2=None, op0=AL.add
                                )
                                nc.vector.tensor_tensor(out=tw, in0=tw, in1=tu, op=AL.mult)
                            nc.vector.tensor_scalar(
                                out=tw, in0=tw, scalar1=c0_, scalar2=None, op0=AL.add
                            )
                            nc.vector.tensor_tensor(out=t_q, in0=tw, in1=tx, op=AL.mult)
                        else:
                            nc.scalar.activation(
                                out=t_q,
                                in_=kf_sb[:, 0:ln],
                                func=mybir.ActivationFunctionType.Tanh,
                                bias=qfT_sb[:, q : q + 1],
                                scale=1.0,
                            )
                        for c0, cn in nchunks:
                            nc.tensor.matmul(
                                out=sg[p : p + 1, c0 : c0 + cn],
                                lhsT=wv_sb,
                                rhs=t_q[:, c0 : c0 + cn],
                                start=True,
                                stop=True,
                                skip_group_check=True,
                                tile_position=(0, p),
                            )
                    stg = spool.tile([128, ln], _F32, tag="stg")
                    nc.vector.tensor_copy(stg, sg)
                    eng = nc.sync if last else nc.gpsimd
                    eng.dma_start(out=s_b[g * 4 : g * 4 + 4, :], in_=stg[0:128:32, :])
                return s_b

            def epilogue(b, s_b, v_b):
                """softmax + attn @ V + output DMA for batch b. Keys beyond
                the valid length were never computed, so no mask is needed
                (exactly matching the reference's masked softmax)."""
                ln = int(lens[b])
                nkc = (ln + 127) // 128
                e_b = softm.tile([QPC, ln], _BF16, tag="e")
                rsum = softm.tile([QPC, 1], _F32, tag="rs")
                nc.scalar.activation(
                    out=e_b, in_=s_b, func=mybir.ActivationFunctionType.Exp, accum_out=rsum
                )
                rinv = softm.tile([QPC, 1], _F32, tag="ri")
                nc.vector.reciprocal(rinv, rsum)

                eT_b = softm.tile([128, nkc * QPC], _BF16, tag="et")
                for g4 in range(0, nkc, 4):
                    gn = min(4, nkc - g4)
                    pst = ps_tail.tile([128, 4 * QPC], _BF16, tag="tail")
                    for j in range(gn):
                        kc = g4 + j
                        r = min(128, ln - kc * 128)
                        nc.tensor.transpose(
                            pst[0:r, j * QPC : (j + 1) * QPC],
                            e_b[:, kc * 128 : kc * 128 + r],
                            ident[0:QPC, 0:QPC],
                        )
                    nc.vector.tensor_copy(
                        eT_b[:, g4 * QPC : (g4 + gn) * QPC], pst[:, 0 : gn * QPC]
                    )

                o_ps = ps_tail.tile([QPC, DV], _F32, tag="tail")
                for kc in range(nkc):
                    r = min(128, ln - kc * 128)
                    nc.tensor.matmul(
                        out=o_ps,
                        lhsT=eT_b[0:r, kc * QPC : (kc + 1) * QPC],
                        rhs=v_b[0:r, kc, :],
                        start=(kc == 0),
                        stop=(kc == nkc - 1),
                    )
                osb = softm.tile([QPC, DV], _F32, tag="o")
                nc.vector.tensor_scalar_mul(osb, o_ps, rinv[:, 0:1])
                nc.sync.dma_start(out=outp[b, :, :], in_=osb)

            # ---- software-pipelined batch loop ---------------------------
            # epilogue(b) is issued after scores(b+1) so ScalarE's exp never
            # blocks the next batch's tanh stream waiting on gather DMAs.
            # Batch order: 2nd-smallest first (short pipeline fill), smallest
            # last (short drain: its gather chain and epilogue set the tail).
            asc = sorted(range(B), key=lambda b: (int(lens[b]), b))
            batch_order = [asc[1]] + asc[2:] + [asc[0]]
            pending = None
            for bi, b in enumerate(batch_order):
                kf_sb, v_b = k_path(b)
                s_b = scores(b, kf_sb, last=(bi == B - 1))
                if pending is not None:
                    epilogue(*pending)
                pending = (b, s_b, v_b)
            epilogue(*pending)

    nc.finalize()
    return nc


def _get_nc(extents):
    key = tuple(int(e) for e in extents)
    if key not in _cached:
        _cached[key] = _build(key)
    return _cached[key]


def kernel(Q, K, V, valid_lengths, W_q, W_k, w_v, _want_trace=False):
    Q = np.asarray(Q, dtype=np.float32)
    K = np.asarray(K, dtype=np.float32)
    V = np.asarray(V, dtype=np.float32)
    vl = np.asarray(valid_lengths).astype(np.int64).reshape(B)
    W_q = np.asarray(W_q, dtype=np.float32)
    W_k = np.asarray(W_k, dtype=np.float32)
    w_v = np.asarray(w_v, dtype=np.float32)

    lens = np.clip(vl, 1, LK)
    extents = np.clip(np.ceil(lens / 128.0).astype(int) * 128, 128, LK)
    nc = _get_nc(lens)

    bf = ml_dtypes.bfloat16
    Kc = np.concatenate([K[b, : extents[b], :] for b in range(B)], axis=0).astype(bf)
    Vc = np.concatenate([V[b, : extents[b], :] for b in range(B)], axis=0).astype(bf)
    Wqb = W_q.astype(bf)
    Wkb = W_k.astype(bf)
    wvb = w_v.reshape(H, 1).astype(bf)
    Qb = Q.astype(bf)

    in_maps = []
    for c in range(N_CORES):
        Qcore = np.concatenate(
            [Qb[b, c * QPC : (c + 1) * QPC, :] for b in range(B)], axis=0
        )
        in_maps.append(
            {
                "Q": Qcore,
                "K": Kc,
                "V": Vc,
                "Wq": Wqb,
                "Wk": Wkb,
                "wv": wvb,
                "ident": np.eye(128, dtype=bf),
            }
        )

    kwargs = {"trace": True} if _want_trace else {}
    res = run_bass_kernel_spmd(nc, in_maps, core_ids=list(range(N_CORES)), **kwargs)
    out = np.empty((B, LQ, DV), dtype=np.float32)
    for c in range(N_CORES):
        oc = res.results[c]["out"]  # (B, QPC, DV)
        for b in range(B):
            out[b, c * QPC : (c + 1) * QPC, :] = oc[b]
    if _want_trace:
        _cached["last_result"] = res
    return out
